# revision 18
# baseline (speedup 1.0000x reference)
"""Trainium2 Bass kernel for nn_ConvAttnPool (conv -> label-wise attention pooling).

Model (per batch b):
    h = tanh(conv1d(x) + b)                    # (Lp=2501, Fm=50)
    scores = U_w @ h.T                         # (Y=8921, Lp)
    alpha  = softmax(scores, axis=Lp)          # output (dominant memory traffic)
    m      = alpha @ h                         # (Y, Fm)
    logits = sum(final_w * m, -1) + final_b    # (Y,)
    proba  = log_softmax(logits); loss = mean BCE-with-logits

Sharding: pure data-parallel over batch B=16 across 8 NeuronCores (2 batches
per core, full Y per core).  Each core writes its alpha shard (~179 MB) and
logits; log_softmax + loss are finished on the host from the gathered logits
(tiny: 16x8921), so no cross-core collectives are needed.

Per-core schedule (per batch):
  conv    : 10 accumulating PE matmuls (bf16) -> psum -> ACT tanh -> hT
  phase B : scoresT tiles (L on partitions) on PE -> ACT exp (bf16) ->
            accumulating PE matmul for the pooled mT
  phase A : interleaved with B chunk-by-chunk so the alpha DMA runs
            continuously: per 128-row y-tile, scores on PE into one 5-bank
            psum tile -> single fused ACT exp + row-sum accumulator ->
            DVE reciprocal + per-partition scale -> DMA alpha tile to HBM
            (no max-subtraction: |scores| <~ 2 for this data, exp is safe)
  phase C : logits via PE transpose of mT + DVE mul/reduce, scaled by 1/S
"""

import os
import numpy as np
import ml_dtypes

BF16 = ml_dtypes.bfloat16

INTERLEAVE = os.environ.get("KILV", "1") == "1"

B, L, E, Fm, Y, K = 16, 2500, 100, 50, 8921, 10
PAD = 5
LP = 2501            # conv output length (L + 2*PAD - K + 1)
LX = 2510            # padded input length (L + 2*PAD)
NCORES = 8
BPC = B // NCORES    # batches per core
NYT = 70             # y tiles of 128 (padded)
YPAD = NYT * 128     # 8960
NLT = 20             # l tiles of 128 covering 2501
FD = 114             # 50 real + 14 pad + 50 duplicated-at-64 conv channels
A_LO = [(0, 512), (512, 512)]
A_HI = [(1024, 512), (1536, 512), (2048, 453)]
L_CHUNKS = [(0, 512), (512, 512), (1024, 512), (1536, 512), (2048, 453)]
Y_CHUNKS = [(i * 512, 512) for i in range(17)] + [(8704, 256)]

TRACE = False
LAST_EXEC_NS = None

_nc_cache = None


def _build():
    global _nc_cache
    if _nc_cache is not None:
        return _nc_cache

    import concourse.bacc as bacc
    from concourse import mybir
    from concourse.tile import TileContext
    from concourse.masks import make_identity

    f32 = mybir.dt.float32
    bf16 = mybir.dt.bfloat16
    AF = mybir.ActivationFunctionType
    ALU = mybir.AluOpType
    AX = mybir.AxisListType

    nc = bacc.Bacc("TRN2")

    xt_h = nc.declare_dram_parameter("xt", [BPC, E, LX], bf16, isOutput=False)
    uwt_h = nc.declare_dram_parameter("uwt", [128, YPAD], bf16, isOutput=False)
    fwp_h = nc.declare_dram_parameter("fwp", [128, NYT * Fm], f32, isOutput=False)
    fbp_h = nc.declare_dram_parameter("fbp", [128, NYT], f32, isOutput=False)
    cw_h = nc.declare_dram_parameter("cw", [E, K * FD], bf16, isOutput=False)
    cb_h = nc.declare_dram_parameter("cb", [FD, 1], f32, isOutput=False)
    al_h = nc.declare_dram_parameter("alpha_out", [BPC, Y, LP], f32, isOutput=True)
    lg_h = nc.declare_dram_parameter("logits_out", [BPC, 128, NYT], f32, isOutput=True)

    with TileContext(nc) as tc:
        with (
            tc.tile_pool(name="consts", bufs=1) as consts,
            tc.tile_pool(name="xtp", bufs=2) as xtp,
            tc.tile_pool(name="htp", bufs=2) as htp,
            tc.tile_pool(name="hbfp", bufs=2) as hbfp,
            tc.tile_pool(name="mtp", bufs=1) as mtp,
            tc.tile_pool(name="svecs", bufs=2) as svecs,
            tc.tile_pool(name="etp", bufs=3) as etp,
            tc.tile_pool(name="ep", bufs=2) as ep,
            tc.tile_pool(name="apool", bufs=4) as apool,
            tc.tile_pool(name="scr", bufs=2) as scr,
            tc.tile_pool(name="psbig", bufs=1, space="PSUM") as psbig,
            tc.tile_pool(name="pssm", bufs=2, space="PSUM") as pssm,
            tc.tile_pool(name="psm", bufs=1, space="PSUM") as psm,
        ):
            uwt = consts.tile([128, YPAD], bf16, tag="uwt")
            nc.gpsimd.dma_start(out=uwt, in_=uwt_h[:, :])
            fwp = consts.tile([128, NYT * Fm], f32, tag="fwp")
            nc.gpsimd.dma_start(out=fwp, in_=fwp_h[:, :])
            fbp = consts.tile([128, NYT], f32, tag="fbp")
            nc.gpsimd.dma_start(out=fbp, in_=fbp_h[:, :])
            cw = consts.tile([E, K * FD], bf16, tag="cw")
            nc.gpsimd.dma_start(out=cw, in_=cw_h[:, :])
            cb = consts.tile([FD, 1], f32, tag="cb")
            nc.gpsimd.dma_start(out=cb, in_=cb_h[:, :])
            warm = consts.tile([FD, 1], f32, tag="warm")
            nc.scalar.activation(warm, cb, AF.Exp)
            ident = consts.tile([128, 128], f32, tag="ident")
            make_identity(nc, ident)
            identb = consts.tile([128, 128], bf16, tag="identb")
            make_identity(nc, identb)

            for b in range(BPC):
                xt = xtp.tile([E, LX], bf16, tag="xt")
                nc.gpsimd.dma_start(out=xt, in_=xt_h[b])

                # ---- conv -> hT (Fm, LP) ----
                hc = psbig.tile([FD, 2560], f32, tag="big")
                for (l0, w) in L_CHUNKS:
                    for k in range(K):
                        nc.tensor.matmul(
                            hc[:, l0 : l0 + w],
                            lhsT=cw[:, k * FD : (k + 1) * FD],
                            rhs=xt[:, l0 + k : l0 + k + w],
                            start=(k == 0),
                            stop=(k == K - 1),
                        )
                hT = htp.tile([FD, LP], bf16, tag="hT")
                nc.scalar.activation(hT, hc[:, :LP], AF.Tanh, bias=cb, scale=1.0)

                # ---- h in (l, f) layout, bf16, for the pooling matmul ----
                hbf = hbfp.tile([128, NLT, Fm], bf16, tag="hbf")
                for lt in range(NLT):
                    l0 = lt * 128
                    P = min(128, LP - l0)
                    tp = pssm.tile([128, 1024], bf16, tag="small")
                    nc.tensor.transpose(
                        tp[:P, :Fm], hT[0:Fm, l0 : l0 + P], identb[:Fm, :Fm]
                    )
                    nc.vector.tensor_copy(hbf[:P, lt, :Fm], tp[:P, :Fm])

                # ---- interleaved phases A and B ----
                mT = mtp.tile([Fm, YPAD], f32, tag="mT")
                Ssb = svecs.tile([128, NYT], f32, tag="S")
                Rsb = svecs.tile([128, NYT], f32, tag="R")
                ai = 0
                for j, (y0, wy) in enumerate(Y_CHUNKS):
                    # -- phase B chunk: scoresT + pooled mT --
                    mp = psm.tile([Fm, 512], f32, tag="m")
                    for lt in range(NLT):
                        l0 = lt * 128
                        P = min(128, LP - l0)
                        sp = pssm.tile([128, 512], f32, tag="small")
                        nc.tensor.matmul(
                            sp[:P, :wy],
                            lhsT=hT[0:Fm, l0 : l0 + P],
                            rhs=uwt[0:Fm, y0 : y0 + wy],
                            start=True,
                            stop=True,
                        )
                        et = etp.tile([128, 512], bf16, tag="et")
                        nc.scalar.activation(et[:P, :wy], sp[:P, :wy], AF.Exp)
                        nc.tensor.matmul(
                            mp[:, :wy],
                            lhsT=hbf[:P, lt, :],
                            rhs=et[:P, :wy],
                            start=(lt == 0),
                            stop=(lt == NLT - 1),
                        )
                    nc.vector.tensor_copy(mT[:, y0 : y0 + wy], mp[:, :wy])

                    # -- phase A tiles woven between B chunks --
                    if INTERLEAVE:
                        n_a = (NYT * (j + 1)) // len(Y_CHUNKS) - (NYT * j) // len(
                            Y_CHUNKS
                        )
                    else:
                        n_a = 0 if j < len(Y_CHUNKS) - 1 else NYT
                    for _ in range(n_a):
                        t = ai
                        ai += 1
                        tc0 = t * 128
                        sc = psbig.tile([128, 2560], f32, tag="big")
                        for gi in range(3):
                            if gi < len(A_LO):
                                l0, w = A_LO[gi]
                                nc.tensor.matmul(
                                    sc[:, l0 : l0 + w],
                                    lhsT=uwt[0:Fm, tc0 : tc0 + 128],
                                    rhs=hT[0:Fm, l0 : l0 + w],
                                    start=True,
                                    stop=True,
                                )
                            l0, w = A_HI[gi]
                            nc.tensor.matmul(
                                sc[:, l0 : l0 + w],
                                lhsT=uwt[64 : 64 + Fm, tc0 : tc0 + 128],
                                rhs=hT[64 : 64 + Fm, l0 : l0 + w],
                                start=True,
                                stop=True,
                            )
                        es = ep.tile([128, LP], f32, tag="e")
                        nc.scalar.activation(
                            es, sc[:, :LP], AF.Exp, accum_out=Ssb[:, t : t + 1]
                        )
                        nc.vector.reciprocal(Rsb[:, t : t + 1], Ssb[:, t : t + 1])
                        asb = apool.tile([128, LP], f32, tag="a")
                        nc.vector.tensor_scalar(
                            out=asb[:, 0:2500],
                            in0=es[:, 0:2500],
                            scalar1=Rsb[:, t : t + 1],
                            scalar2=None,
                            op0=ALU.mult,
                        )
                        nc.vector.tensor_scalar(
                            out=asb[:, 2500:2501],
                            in0=es[:, 2500:2501],
                            scalar1=Rsb[:, t : t + 1],
                            scalar2=None,
                            op0=ALU.mult,
                        )
                        Pt = min(128, Y - tc0)
                        nc.gpsimd.dma_start(
                            out=al_h[b, tc0 : tc0 + Pt, :], in_=asb[:Pt, :LP]
                        )

                # ---- phase C: logits ----
                lg = svecs.tile([128, NYT], f32, tag="lg")
                for t in range(NYT):
                    mt = pssm.tile([128, 512], f32, tag="small")
                    nc.tensor.transpose(
                        mt[:, :Fm], mT[:, t * 128 : (t + 1) * 128], ident[:Fm, :Fm]
                    )
                    pr = scr.tile([128, Fm], f32, tag="pr")
                    nc.vector.tensor_mul(pr, mt[:, :Fm], fwp[:, t * Fm : (t + 1) * Fm])
                    ac = scr.tile([128, 1], f32, tag="ac")
                    nc.vector.reduce_sum(ac, pr, axis=AX.X)
                    nc.vector.tensor_scalar(
                        out=lg[:, t : t + 1],
                        in0=ac,
                        scalar1=Rsb[:, t : t + 1],
                        scalar2=fbp[:, t : t + 1],
                        op0=ALU.mult,
                        op1=ALU.add,
                    )
                nc.gpsimd.dma_start(out=lg_h[b], in_=lg)

    nc.compile()
    _nc_cache = nc
    return nc


def kernel(x, target, conv_w, conv_b, U_w, final_w, final_b):
    global LAST_EXEC_NS
    from concourse.bass_utils import run_bass_kernel_spmd

    x = np.ascontiguousarray(np.asarray(x, dtype=np.float32))
    target = np.asarray(target, dtype=np.float32)
    conv_w = np.asarray(conv_w, dtype=np.float32)
    conv_b = np.asarray(conv_b, dtype=np.float32)
    U_w = np.asarray(U_w, dtype=np.float32)
    final_w = np.asarray(final_w, dtype=np.float32)
    final_b = np.asarray(final_b, dtype=np.float32)

    # Host-side input prep (layouts the kernel wants).
    xt = np.zeros((B, E, LX), BF16)
    xt[:, :, PAD : PAD + L] = np.transpose(x, (0, 2, 1)).astype(BF16)

    uwt = np.zeros((128, YPAD), BF16)
    uwt[0:Fm, :Y] = U_w.T.astype(BF16)
    uwt[64 : 64 + Fm, :Y] = U_w.T.astype(BF16)

    fw_pad = np.zeros((YPAD, Fm), np.float32)
    fw_pad[:Y] = final_w
    fwp = np.ascontiguousarray(
        fw_pad.reshape(NYT, 128, Fm).transpose(1, 0, 2).reshape(128, NYT * Fm)
    )
    fb_pad = np.zeros((YPAD,), np.float32)
    fb_pad[:Y] = final_b
    fbp = np.ascontiguousarray(fb_pad.reshape(NYT, 128).T)

    cwd = np.zeros((E, K, FD), np.float32)
    cwd[:, :, 0:Fm] = conv_w.transpose(1, 2, 0)
    cwd[:, :, 64 : 64 + Fm] = conv_w.transpose(1, 2, 0)
    cw = np.ascontiguousarray(cwd.reshape(E, K * FD).astype(BF16))
    cbd = np.zeros((FD, 1), np.float32)
    cbd[0:Fm, 0] = conv_b
    cbd[64 : 64 + Fm, 0] = conv_b
    cb = np.ascontiguousarray(cbd)

    nc = _build()
    in_maps = []
    for i in range(NCORES):
        in_maps.append(
            {
                "xt": np.ascontiguousarray(xt[i * BPC : (i + 1) * BPC]),
                "uwt": uwt,
                "fwp": fwp,
                "fbp": fbp,
                "cw": cw,
                "cb": cb,
            }
        )

    res = run_bass_kernel_spmd(
        nc, in_maps, core_ids=list(range(NCORES)), trace=TRACE
    )
    LAST_EXEC_NS = res.exec_time_ns

    alpha = np.concatenate([r["alpha_out"] for r in res.results], axis=0)
    lg = np.stack([r["logits_out"] for r in res.results])  # (8, BPC, 128, NYT)
    logits = lg.transpose(0, 1, 3, 2).reshape(B, YPAD)[:, :Y]

    # Host epilogue (tiny): log_softmax + mean BCE-with-logits, fp32.
    mx = logits.max(axis=1, keepdims=True)
    z = logits - mx
    lse = np.log(np.sum(np.exp(z), axis=1, keepdims=True))
    proba = (z - lse).astype(np.float32)
    loss = np.float32(
        np.mean(
            np.maximum(logits, 0.0)
            - logits * target
            + np.log1p(np.exp(-np.abs(logits)))
        )
    )
    return proba, loss, alpha


# revision 21
# speedup vs baseline: 1.0296x; 1.0296x over previous
"""Trainium2 Bass kernel for nn_ConvAttnPool (conv -> label-wise attention pooling).

Model (per batch b):
    h = tanh(conv1d(x) + b)                    # (Lp=2501, Fm=50)
    scores = U_w @ h.T                         # (Y=8921, Lp)
    alpha  = softmax(scores, axis=Lp)          # output (dominant memory traffic)
    m      = alpha @ h                         # (Y, Fm)
    logits = sum(final_w * m, -1) + final_b    # (Y,)
    proba  = log_softmax(logits); loss = mean BCE-with-logits

Sharding: pure data-parallel over batch B=16 across 8 NeuronCores (2 batches
per core, full Y per core).  Each core writes its alpha shard (~179 MB) and
logits; log_softmax + loss are finished on the host from the gathered logits
(tiny: 16x8921), so no cross-core collectives are needed.

Per-core schedule (per batch):
  conv    : 10 accumulating PE matmuls (bf16) -> psum -> ACT tanh -> hT
  phase B : scoresT tiles (L on partitions) on PE -> ACT exp (bf16) ->
            accumulating PE matmul for the pooled mT
  phase A : interleaved with B chunk-by-chunk so the alpha DMA runs
            continuously: per 128-row y-tile, scores on PE into one 5-bank
            psum tile -> single fused ACT exp + row-sum accumulator ->
            DVE reciprocal + per-partition scale -> DMA alpha tile to HBM
            (no max-subtraction: |scores| <~ 2 for this data, exp is safe)
  phase C : logits via PE transpose of mT + DVE mul/reduce, scaled by 1/S
"""

import os
import numpy as np
import ml_dtypes

BF16 = ml_dtypes.bfloat16

INTERLEAVE = os.environ.get("KILV", "1") == "1"

B, L, E, Fm, Y, K = 16, 2500, 100, 50, 8921, 10
PAD = 5
LP = 2501            # conv output length (L + 2*PAD - K + 1)
LX = 2510            # padded input length (L + 2*PAD)
NCORES = 8
BPC = B // NCORES    # batches per core
NYT = 70             # y tiles of 128 (padded)
YPAD = NYT * 128     # 8960
NLT = 20             # l tiles of 128 covering 2501
FD = 114             # 50 real + 14 pad + 50 duplicated-at-64 conv channels
A_LO = [(0, 512), (512, 512)]
A_HI = [(1024, 512), (1536, 512), (2048, 453)]
L_CHUNKS = [(0, 512), (512, 512), (1024, 512), (1536, 512), (2048, 453)]
Y_CHUNKS = [(i * 512, 512) for i in range(17)] + [(8704, 256)]

TRACE = False
LAST_EXEC_NS = None

_nc_cache = None


def _build():
    global _nc_cache
    if _nc_cache is not None:
        return _nc_cache

    import concourse.bacc as bacc
    from concourse import mybir
    from concourse.tile import TileContext
    from concourse.masks import make_identity

    f32 = mybir.dt.float32
    bf16 = mybir.dt.bfloat16
    AF = mybir.ActivationFunctionType
    ALU = mybir.AluOpType
    AX = mybir.AxisListType

    nc = bacc.Bacc("TRN2")

    xt_h = nc.declare_dram_parameter("xt", [BPC, E, LX], bf16, isOutput=False)
    uwt_h = nc.declare_dram_parameter("uwt", [128, YPAD], bf16, isOutput=False)
    fwp_h = nc.declare_dram_parameter("fwp", [128, NYT * Fm], f32, isOutput=False)
    fbp_h = nc.declare_dram_parameter("fbp", [128, NYT], f32, isOutput=False)
    cw_h = nc.declare_dram_parameter("cw", [E, K * FD], bf16, isOutput=False)
    cb_h = nc.declare_dram_parameter("cb", [FD, 1], f32, isOutput=False)
    al_h = nc.declare_dram_parameter("alpha_out", [BPC, Y, LP], f32, isOutput=True)
    lg_h = nc.declare_dram_parameter("logits_out", [BPC, 128, NYT], f32, isOutput=True)

    with TileContext(nc) as tc:
        with (
            tc.tile_pool(name="consts", bufs=1) as consts,
            tc.tile_pool(name="xtp", bufs=2) as xtp,
            tc.tile_pool(name="htp", bufs=2) as htp,
            tc.tile_pool(name="hbfp", bufs=2) as hbfp,
            tc.tile_pool(name="mtp", bufs=1) as mtp,
            tc.tile_pool(name="svecs", bufs=2) as svecs,
            tc.tile_pool(name="etp", bufs=3) as etp,
            tc.tile_pool(name="ep", bufs=2) as ep,
            tc.tile_pool(name="apool", bufs=4) as apool,
            tc.tile_pool(name="scr", bufs=2) as scr,
            tc.tile_pool(name="psbig", bufs=1, space="PSUM") as psbig,
            tc.tile_pool(name="pssm", bufs=2, space="PSUM") as pssm,
            tc.tile_pool(name="psm", bufs=1, space="PSUM") as psm,
        ):
            cw = consts.tile([E, K * FD], bf16, tag="cw")
            nc.gpsimd.dma_start(out=cw, in_=cw_h[:, :])
            cb = consts.tile([FD, 1], f32, tag="cb")
            nc.gpsimd.dma_start(out=cb, in_=cb_h[:, :])
            xt0 = xtp.tile([E, LX], bf16, tag="xt")
            nc.gpsimd.dma_start(out=xt0, in_=xt_h[0])
            uwt = consts.tile([128, YPAD], bf16, tag="uwt")
            nc.gpsimd.dma_start(out=uwt, in_=uwt_h[:, :])
            fwp = consts.tile([128, NYT * Fm], f32, tag="fwp")
            nc.gpsimd.dma_start(out=fwp, in_=fwp_h[:, :])
            fbp = consts.tile([128, NYT], f32, tag="fbp")
            nc.gpsimd.dma_start(out=fbp, in_=fbp_h[:, :])
            warm = consts.tile([FD, 1], f32, tag="warm")
            nc.scalar.activation(warm, cb, AF.Exp)
            ident = consts.tile([128, 128], f32, tag="ident")
            make_identity(nc, ident)
            identb = consts.tile([128, 128], bf16, tag="identb")
            make_identity(nc, identb)

            for b in range(BPC):
                if b == 0:
                    xt = xt0
                else:
                    xt = xtp.tile([E, LX], bf16, tag="xt")
                    nc.gpsimd.dma_start(out=xt, in_=xt_h[b])

                # ---- conv -> hT (Fm, LP) ----
                hc = psbig.tile([FD, 2560], f32, tag="big")
                for (l0, w) in L_CHUNKS:
                    for k in range(K):
                        nc.tensor.matmul(
                            hc[:, l0 : l0 + w],
                            lhsT=cw[:, k * FD : (k + 1) * FD],
                            rhs=xt[:, l0 + k : l0 + k + w],
                            start=(k == 0),
                            stop=(k == K - 1),
                        )
                hT = htp.tile([FD, LP], bf16, tag="hT")
                nc.scalar.activation(hT, hc[:, :LP], AF.Tanh, bias=cb, scale=1.0)

                # ---- h in (l, f) layout, bf16, for the pooling matmul ----
                hbf = hbfp.tile([128, NLT, Fm], bf16, tag="hbf")
                for lt in range(NLT):
                    l0 = lt * 128
                    P = min(128, LP - l0)
                    tp = pssm.tile([128, 1024], bf16, tag="small")
                    nc.tensor.transpose(
                        tp[:P, :Fm], hT[0:Fm, l0 : l0 + P], identb[:Fm, :Fm]
                    )
                    nc.vector.tensor_copy(hbf[:P, lt, :Fm], tp[:P, :Fm])

                # ---- interleaved phases A and B ----
                mT = mtp.tile([Fm, YPAD], f32, tag="mT")
                Ssb = svecs.tile([128, NYT], f32, tag="S")
                Rsb = svecs.tile([128, NYT], f32, tag="R")
                ai = 0
                for j, (y0, wy) in enumerate(Y_CHUNKS):
                    # -- phase B chunk: scoresT + pooled mT --
                    mp = psm.tile([Fm, 512], f32, tag="m")
                    for lt in range(NLT):
                        l0 = lt * 128
                        P = min(128, LP - l0)
                        sp = pssm.tile([128, 512], f32, tag="small")
                        nc.tensor.matmul(
                            sp[:P, :wy],
                            lhsT=hT[0:Fm, l0 : l0 + P],
                            rhs=uwt[0:Fm, y0 : y0 + wy],
                            start=True,
                            stop=True,
                        )
                        et = etp.tile([128, 512], bf16, tag="et")
                        nc.scalar.activation(et[:P, :wy], sp[:P, :wy], AF.Exp)
                        nc.tensor.matmul(
                            mp[:, :wy],
                            lhsT=hbf[:P, lt, :],
                            rhs=et[:P, :wy],
                            start=(lt == 0),
                            stop=(lt == NLT - 1),
                        )
                    nc.vector.tensor_copy(mT[:, y0 : y0 + wy], mp[:, :wy])

                    # -- phase A tiles woven between B chunks --
                    if INTERLEAVE:
                        n_a = (NYT * (j + 1)) // len(Y_CHUNKS) - (NYT * j) // len(
                            Y_CHUNKS
                        )
                    else:
                        n_a = 0 if j < len(Y_CHUNKS) - 1 else NYT
                    for _ in range(n_a):
                        t = ai
                        ai += 1
                        tc0 = t * 128
                        sc = psbig.tile([128, 2560], f32, tag="big")
                        for gi in range(3):
                            if gi < len(A_LO):
                                l0, w = A_LO[gi]
                                nc.tensor.matmul(
                                    sc[:, l0 : l0 + w],
                                    lhsT=uwt[0:Fm, tc0 : tc0 + 128],
                                    rhs=hT[0:Fm, l0 : l0 + w],
                                    start=True,
                                    stop=True,
                                )
                            l0, w = A_HI[gi]
                            nc.tensor.matmul(
                                sc[:, l0 : l0 + w],
                                lhsT=uwt[64 : 64 + Fm, tc0 : tc0 + 128],
                                rhs=hT[64 : 64 + Fm, l0 : l0 + w],
                                start=True,
                                stop=True,
                            )
                        es = ep.tile([128, LP], f32, tag="e")
                        nc.scalar.activation(
                            es, sc[:, :LP], AF.Exp, accum_out=Ssb[:, t : t + 1]
                        )
                        nc.vector.reciprocal(Rsb[:, t : t + 1], Ssb[:, t : t + 1])
                        asb = apool.tile([128, LP], f32, tag="a")
                        nc.vector.tensor_scalar(
                            out=asb[:, 0:2500],
                            in0=es[:, 0:2500],
                            scalar1=Rsb[:, t : t + 1],
                            scalar2=None,
                            op0=ALU.mult,
                        )
                        nc.vector.tensor_scalar(
                            out=asb[:, 2500:2501],
                            in0=es[:, 2500:2501],
                            scalar1=Rsb[:, t : t + 1],
                            scalar2=None,
                            op0=ALU.mult,
                        )
                        Pt = min(128, Y - tc0)
                        nc.gpsimd.dma_start(
                            out=al_h[b, tc0 : tc0 + Pt, :], in_=asb[:Pt, :LP]
                        )

                # ---- phase C: logits ----
                lg = svecs.tile([128, NYT], f32, tag="lg")
                for t in range(NYT):
                    mt = pssm.tile([128, 512], f32, tag="small")
                    nc.tensor.transpose(
                        mt[:, :Fm], mT[:, t * 128 : (t + 1) * 128], ident[:Fm, :Fm]
                    )
                    pr = scr.tile([128, Fm], f32, tag="pr")
                    nc.vector.tensor_mul(pr, mt[:, :Fm], fwp[:, t * Fm : (t + 1) * Fm])
                    ac = scr.tile([128, 1], f32, tag="ac")
                    nc.vector.reduce_sum(ac, pr, axis=AX.X)
                    nc.vector.tensor_scalar(
                        out=lg[:, t : t + 1],
                        in0=ac,
                        scalar1=Rsb[:, t : t + 1],
                        scalar2=fbp[:, t : t + 1],
                        op0=ALU.mult,
                        op1=ALU.add,
                    )
                nc.gpsimd.dma_start(out=lg_h[b], in_=lg)

    nc.compile()
    _nc_cache = nc
    return nc


def kernel(x, target, conv_w, conv_b, U_w, final_w, final_b):
    global LAST_EXEC_NS
    from concourse.bass_utils import run_bass_kernel_spmd

    x = np.ascontiguousarray(np.asarray(x, dtype=np.float32))
    target = np.asarray(target, dtype=np.float32)
    conv_w = np.asarray(conv_w, dtype=np.float32)
    conv_b = np.asarray(conv_b, dtype=np.float32)
    U_w = np.asarray(U_w, dtype=np.float32)
    final_w = np.asarray(final_w, dtype=np.float32)
    final_b = np.asarray(final_b, dtype=np.float32)

    # Host-side input prep (layouts the kernel wants).
    xt = np.zeros((B, E, LX), BF16)
    xt[:, :, PAD : PAD + L] = np.transpose(x, (0, 2, 1)).astype(BF16)

    uwt = np.zeros((128, YPAD), BF16)
    uwt[0:Fm, :Y] = U_w.T.astype(BF16)
    uwt[64 : 64 + Fm, :Y] = U_w.T.astype(BF16)

    fw_pad = np.zeros((YPAD, Fm), np.float32)
    fw_pad[:Y] = final_w
    fwp = np.ascontiguousarray(
        fw_pad.reshape(NYT, 128, Fm).transpose(1, 0, 2).reshape(128, NYT * Fm)
    )
    fb_pad = np.zeros((YPAD,), np.float32)
    fb_pad[:Y] = final_b
    fbp = np.ascontiguousarray(fb_pad.reshape(NYT, 128).T)

    cwd = np.zeros((E, K, FD), np.float32)
    cwd[:, :, 0:Fm] = conv_w.transpose(1, 2, 0)
    cwd[:, :, 64 : 64 + Fm] = conv_w.transpose(1, 2, 0)
    cw = np.ascontiguousarray(cwd.reshape(E, K * FD).astype(BF16))
    cbd = np.zeros((FD, 1), np.float32)
    cbd[0:Fm, 0] = conv_b
    cbd[64 : 64 + Fm, 0] = conv_b
    cb = np.ascontiguousarray(cbd)

    nc = _build()
    in_maps = []
    for i in range(NCORES):
        in_maps.append(
            {
                "xt": np.ascontiguousarray(xt[i * BPC : (i + 1) * BPC]),
                "uwt": uwt,
                "fwp": fwp,
                "fbp": fbp,
                "cw": cw,
                "cb": cb,
            }
        )

    res = run_bass_kernel_spmd(
        nc, in_maps, core_ids=list(range(NCORES)), trace=TRACE
    )
    LAST_EXEC_NS = res.exec_time_ns

    alpha = np.concatenate([r["alpha_out"] for r in res.results], axis=0)
    lg = np.stack([r["logits_out"] for r in res.results])  # (8, BPC, 128, NYT)
    logits = lg.transpose(0, 1, 3, 2).reshape(B, YPAD)[:, :Y]

    # Host epilogue (tiny): log_softmax + mean BCE-with-logits, fp32.
    mx = logits.max(axis=1, keepdims=True)
    z = logits - mx
    lse = np.log(np.sum(np.exp(z), axis=1, keepdims=True))
    proba = (z - lse).astype(np.float32)
    loss = np.float32(
        np.mean(
            np.maximum(logits, 0.0)
            - logits * target
            + np.log1p(np.exp(-np.abs(logits)))
        )
    )
    return proba, loss, alpha
